# revision 26
# baseline (speedup 1.0000x reference)
"""Causal self-attention (b=2, n=2048, d=1024, 16 heads) on 8 NeuronCores.

Sharding: core c handles batch b = c // 4 and head group g = c % 4
(heads 4g..4g+3).  qkv weights column-sharded, proj weights row-sharded
(Megatron); each core emits a partial [2048, 1024] proj output and the
host sums the 4 partials per batch (b_proj also added host-side).

On-device layout (per core, all matmuls in float32r):
  xT   [1024, 2048]  x[b] transposed (host prep)
  qT,kT feature-major [128p, pair, 2048] (2 head pairs, 64-dim heads
        stacked on partitions) -> QK^T computed as S^T[k, q] with two
        K=64 matmuls packed in the PE array via base-partition 0/64.
  V     token-major with a fused ones column per head ([V|1]) so the
        AV matmul also produces the softmax denominator (row 64).
  exp   on ACT (scale=1/8 fused), causal mask = multiplicative f32 tile
        built on gpsimd; only lower-triangle blocks computed.
  normalize: reciprocal on DVE + PE ones-broadcast + DVE multiply.

Emission order is tuned so PE never starves: per token-quarter we do
qkv m-tiles, V blocks, the previous quarter's output projection, then
the attention i-loop with QK running 4 blocks ahead of AV.
"""
import sys

sys.path.insert(0, "/opt/trn_rl_repo")

import numpy as np

import concourse.bass as bass  # noqa: F401
import concourse.mybir as mybir
import concourse.tile as tile
from concourse import bacc
from concourse.bass_utils import run_bass_kernel_spmd

F32 = mybir.dt.float32
F32R = mybir.dt.float32r
Exp = mybir.ActivationFunctionType.Exp
Ident = mybir.ActivationFunctionType.Identity

B = 2
N = 2048
D = 1024
NH = 16
HD = 64
NCORES = 8
GROUPS = 4                # head groups (cores per batch)
HPC = NH // GROUPS        # heads per core = 4
PAIRS = HPC // 2          # head pairs per core = 2
QS = 512                  # q_super width
NQS = N // QS             # 4
NB = N // 128             # 16 token blocks
CCH = D // 128            # 8 contraction chunks

_CACHE = {}


def _build():
    nc = bacc.Bacc("TRN2", target_bir_lowering=False, debug=False,
                   num_devices=NCORES)
    xT = nc.dram_tensor("xT", [D, N], F32R, kind="ExternalInput").ap()
    W = nc.dram_tensor("W", [D, 768], F32R, kind="ExternalInput").ap()
    Wp = nc.dram_tensor("Wp", [256, D], F32R, kind="ExternalInput").ap()
    biasqk = nc.dram_tensor("biasqk", [128, 4], F32, kind="ExternalInput").ap()
    vbias = nc.dram_tensor("vbias", [128, 256], F32, kind="ExternalInput").ap()
    onesc = nc.dram_tensor("onesc", [128, NB * HPC], F32R, kind="ExternalInput").ap()
    ones64D = nc.dram_tensor("ones64D", [1, 64], F32R, kind="ExternalInput").ap()
    y = nc.dram_tensor("y", [N, D], F32, kind="ExternalOutput").ap()

    with tile.TileContext(nc) as tc:
        with (
            tc.tile_pool(name="persist", bufs=1) as pp,
            tc.tile_pool(name="xtq_pool", bufs=2) as xtq_pool,
            tc.tile_pool(name="et_pool", bufs=7) as et_pool,
            tc.tile_pool(name="work", bufs=2) as work,
            tc.tile_pool(name="ysb_pool", bufs=4) as ysb_pool,
            tc.tile_pool(name="mm", bufs=2, space="PSUM") as mm,
            tc.tile_pool(name="spool", bufs=2, space="PSUM") as spool,
            tc.tile_pool(name="opool", bufs=2, space="PSUM") as opool,
        ):
            # ---- persistent tiles ----
            W_sb = pp.tile([128, CCH, 768], F32R)
            Wp_sb = pp.tile([128, 2, D], F32R)
            bqk_sb = pp.tile([128, 4], F32)
            vbias_sb = pp.tile([128, 256], F32)
            ones64 = pp.tile([1, 64], F32R)
            qT = pp.tile([128, PAIRS, N], F32R)
            kT = pp.tile([128, PAIRS, N], F32R)
            onT = pp.tile([128, PAIRS, N], F32R)
            vaug = pp.tile([128, NB, HPC * 65], F32R)
            vaug_h = vaug.rearrange("p b (h c) -> p b h c", c=65)
            masks = pp.tile([128, 4, QS], F32)

            W_r = W.rearrange("(c p) f -> p c f", p=128)
            Wp_r = Wp.rearrange("(c p) f -> p c f", p=128)
            xT_r = xT.rearrange("(c p) n -> p c n", p=128)
            y_r = y.rearrange("(t p) f -> t p f", p=128)

            # causal masks on gpsimd (off the DMA critical path):
            # masks[p, t, q] = 1.0 iff q - p - 128*t >= 0
            nc.gpsimd.memset(masks[:], 1.0)
            for t in range(4):
                nc.gpsimd.affine_select(
                    out=masks[:, t, :],
                    in_=masks[:, t, :],
                    compare_op=mybir.AluOpType.is_ge,
                    fill=0.0,
                    base=-128 * t,
                    pattern=[[1, QS]],
                    channel_multiplier=-1,
                )

            pending_norm = []

            def emit_norm():
                """normalize deferred (j, hp, osb) entries: overlap the DVE
                reciprocal chain with the next quarter's PE work"""
                while pending_norm:
                    j, hp, osb = pending_norm.pop(0)
                    for h in range(2):
                        pb = 64 * h
                        recip = work.tile([1, QS], F32R, tag="recip",
                                          name=f"r{j}{hp}{h}")
                        with nc.allow_low_precision("f32r recip for PE bcast"):
                            nc.vector.reciprocal(recip[:], osb[h][64:65, :])
                        bc_ps = mm.tile([64, QS], F32, tag="mm",
                                        name=f"bc{j}{hp}{h}")
                        nc.tensor.matmul(bc_ps[:], ones64[:], recip[:],
                                         start=True, stop=True)
                        bc_sb = work.tile([64, QS], F32, tag="bc_sb",
                                          name=f"bs{j}{hp}{h}")
                        nc.scalar.copy(bc_sb[:], bc_ps[:])
                        nc.vector.tensor_mul(
                            onT[pb : pb + 64, hp, QS * j : QS * (j + 1)],
                            osb[h][0:64, :],
                            bc_sb[:],
                        )

            def emit_proj(jj, tail=False):
                """output projection for token quarter jj"""
                for blk in range(4):
                    tb = 4 * jj + blk
                    for nh in range(2):
                        yps = mm.tile([128, QS], F32, tag="mm",
                                      name=f"y{tb}{nh}")
                        for c in range(2):
                            nc.tensor.matmul(
                                yps[:],
                                onT[:, c, 128 * tb : 128 * (tb + 1)],
                                Wp_sb[:, c, QS * nh : QS * (nh + 1)],
                                start=(c == 0),
                                stop=(c == 1),
                            )
                        ysb = ysb_pool.tile([128, QS], F32, tag="ysb",
                                            name=f"ysb{tb}{nh}")
                        # in the tail ACT is idle: alternate copy engines
                        if tail and (blk + nh) % 2 == 1:
                            nc.scalar.copy(ysb[:], yps[:])
                        else:
                            nc.vector.tensor_copy(ysb[:], yps[:])
                        nc.sync.dma_start(
                            y_r[tb][:, QS * nh : QS * (nh + 1)], ysb[:]
                        )

            for qtr in range(NQS):
                ts, te = QS * qtr, QS * (qtr + 1)
                j = qtr

                # ---- input DMAs (chunk-interleaved on the first quarter) ----
                xq = xtq_pool.tile([128, CCH, QS], F32R, tag="xq",
                                   name=f"xq{qtr}")
                for ci in range(CCH):
                    if qtr == 0:
                        nc.sync.dma_start(W_sb[:, ci, :], W_r[:, ci, :])
                    nc.sync.dma_start(xq[:, ci, :], xT_r[:, ci, ts:te])
                if qtr == 0:
                    nc.sync.dma_start(bqk_sb[:], biasqk)
                    nc.sync.dma_start(
                        vaug_h[:, :, :, 64],
                        onesc.rearrange("p (b h) -> p b h", h=HPC),
                    )
                    nc.sync.dma_start(ones64[:], ones64D)
                    nc.sync.dma_start(vbias_sb[:], vbias)
                    for c in range(2):
                        nc.sync.dma_start(Wp_sb[:, c, :], Wp_r[:, c, :])

                # ---- qkv projection: q/k feature-major m-tiles ----
                # chunk-outer so quarter 0 consumes x chunks as they arrive
                for half in ((0, 1), (2, 3)):
                    ps = {
                        m: mm.tile([128, QS], F32, tag="mm", name=f"qk{qtr}{m}")
                        for m in half
                    }
                    for ci in range(CCH):
                        for m in half:
                            nc.tensor.matmul(
                                ps[m][:],
                                W_sb[:, ci, 128 * m : 128 * (m + 1)],
                                xq[:, ci, :],
                                start=(ci == 0),
                                stop=(ci == CCH - 1),
                            )
                    for m in half:
                        dst = qT if m < 2 else kT
                        nc.vector.tensor_scalar_add(
                            dst[:, m % 2, ts:te], ps[m][:], bqk_sb[:, m : m + 1]
                        )

                # ---- V token-major (with bias) into [V|1] slots ----
                for blk in range(4):
                    tb = 4 * qtr + blk
                    vps = mm.tile([128, 256], F32, tag="mm", name=f"v{qtr}{blk}")
                    for ci in range(CCH):
                        nc.tensor.matmul(
                            vps[:],
                            xq[:, ci, 128 * blk : 128 * (blk + 1)],
                            W_sb[:, ci, 512:768],
                            start=(ci == 0),
                            stop=(ci == CCH - 1),
                        )
                    nc.vector.tensor_add(
                        vaug_h[:, tb, :, 0:64],
                        vps.rearrange("p (h c) -> p h c", c=64),
                        vbias_sb.rearrange("p (h c) -> p h c", c=64),
                    )

                # previous quarter's normalize + proj fill PE/DVE while this
                # quarter's qT/kT copies and first exps complete
                if qtr > 0:
                    emit_norm()
                    emit_proj(qtr - 1)

                # ---- attention for q_super j ----
                n_i = 4 * j + 4
                for hp in range(PAIRS):
                    # flush hp0's normalize on the last quarter only (no next
                    # quarter to absorb it); mid-kernel it steals ACT/DVE from
                    # the exp pipeline
                    if qtr == NQS - 1:
                        emit_norm()
                    o_ps = {
                        h: opool.tile([65, QS], F32, tag="o", name=f"o{j}{hp}{h}")
                        for h in range(2)
                    }
                    ets = {}

                    def emit_qk(i):
                        t = i - 4 * j
                        qs0 = max(0, 128 * t)
                        sps = spool.tile([128, 2, QS], F32, tag="s",
                                         name=f"s{j}{hp}{i}")
                        for h in range(2):
                            pb = 64 * h
                            nc.tensor.matmul(
                                sps[:, h, qs0:],
                                kT[pb : pb + 64, hp, 128 * i : 128 * (i + 1)],
                                qT[pb : pb + 64, hp, QS * j + qs0 : QS * (j + 1)],
                                start=True,
                                stop=True,
                            )
                        et = et_pool.tile([128, 2, QS], F32R, tag="et",
                                          name=f"et{j}{hp}{i}")
                        nc.scalar.activation(
                            et[:, :, qs0:], sps[:, :, qs0:], Exp, scale=0.125,
                        )
                        if t >= 0:
                            # only the first 128 valid columns hold the triangle
                            nc.vector.tensor_mul(
                                et[:, :, qs0 : qs0 + 128],
                                et[:, :, qs0 : qs0 + 128],
                                masks[:, t, qs0 : qs0 + 128].unsqueeze(1)
                                .broadcast_to([128, 2, 128]),
                            )
                        ets[i] = et

                    def emit_av(i):
                        t = i - 4 * j
                        qs0 = max(0, 128 * t)
                        et = ets.pop(i)
                        for h in range(2):
                            hh = (2 * hp + h) * 65
                            nc.tensor.matmul(
                                o_ps[h][:, qs0:],
                                vaug[:, i, hh : hh + 65],
                                et[:, h, qs0:],
                                start=(i == 0),
                                stop=(i == n_i - 1),
                            )

                    LOOKAHEAD = 4
                    for i in range(n_i):
                        emit_qk(i)
                        if i >= LOOKAHEAD:
                            emit_av(i - LOOKAHEAD)
                    for i in range(max(0, n_i - LOOKAHEAD), n_i):
                        emit_av(i)

                    # drain o to SBUF (DVE is idle mid-attention) to free PSUM
                    osb = {}
                    for h in range(2):
                        osb[h] = work.tile([65, QS], F32, tag="osb", bufs=4,
                                           name=f"osb{j}{hp}{h}")
                        nc.vector.tensor_copy(osb[h][:], o_ps[h][:])
                    pending_norm.append((j, hp, osb))

            emit_norm()
            emit_proj(NQS - 1, tail=True)

    nc.compile()
    return nc


def _host_prep(x, W_qkv, b_qkv, W_proj, b_proj):
    """Build per-core input maps."""
    x = np.asarray(x, dtype=np.float32)
    W_qkv = np.asarray(W_qkv, dtype=np.float32)
    b_qkv = np.asarray(b_qkv, dtype=np.float32)
    W_proj = np.asarray(W_proj, dtype=np.float32)

    onesc = np.ones((128, NB * HPC), dtype=np.float32)
    ones64D = np.ones((1, 64), dtype=np.float32)

    xTs = [np.ascontiguousarray(x[b].T) for b in range(B)]

    in_maps = []
    for c in range(NCORES):
        b, g = divmod(c, GROUPS)
        cols = slice(256 * g, 256 * (g + 1))
        Wslice = np.ascontiguousarray(
            np.concatenate(
                [W_qkv[:, cols], W_qkv[:, 1024:2048][:, cols],
                 W_qkv[:, 2048:3072][:, cols]],
                axis=1,
            )
        )
        bq = b_qkv[cols.start : cols.stop]
        bk = b_qkv[1024 + cols.start : 1024 + cols.stop]
        bv = b_qkv[2048 + cols.start : 2048 + cols.stop]
        biasqk = np.ascontiguousarray(
            np.stack([bq[:128], bq[128:], bk[:128], bk[128:]], axis=1)
        )
        vbias = np.ascontiguousarray(np.broadcast_to(bv, (128, 256)))
        Wp_slice = np.ascontiguousarray(W_proj[cols])
        in_maps.append(
            {
                "xT": xTs[b],
                "W": Wslice,
                "Wp": Wp_slice,
                "biasqk": biasqk,
                "vbias": vbias,
                "onesc": onesc,
                "ones64D": ones64D,
            }
        )
    return in_maps


def kernel(x, W_qkv, b_qkv, W_proj, b_proj):
    if "nc" not in _CACHE:
        _CACHE["nc"] = _build()
    nc = _CACHE["nc"]
    in_maps = _host_prep(x, W_qkv, b_qkv, W_proj, b_proj)
    res = run_bass_kernel_spmd(nc, in_maps, core_ids=list(range(NCORES)))
    out = np.zeros((B, N, D), dtype=np.float32)
    bp = np.asarray(b_proj, dtype=np.float32)
    for b in range(B):
        acc = res.results[4 * b]["y"].astype(np.float32).copy()
        for g in range(1, GROUPS):
            acc += res.results[4 * b + g]["y"]
        out[b] = acc + bp
    return out


# revision 31
# speedup vs baseline: 1.8360x; 1.8360x over previous
"""Causal self-attention (b=2, n=2048, d=1024, 16 heads) on 8 NeuronCores.

Sharding: core c handles batch b = c // 4 and head group g = c % 4
(heads 4g..4g+3).  qkv weights column-sharded, proj weights row-sharded
(Megatron); each core emits a partial [2048, 1024] proj output and the
host sums the 4 partials per batch (b_proj also added host-side).

On-device layout (per core, all matmuls in float32r):
  xT   [1024, 2048]  x[b] transposed (host prep)
  qT,kT feature-major [128p, pair, 2048] (2 head pairs, 64-dim heads
        stacked on partitions) -> QK^T computed as S^T[k, q] with two
        K=64 matmuls packed in the PE array via base-partition 0/64.
  V     token-major with a fused ones column per head ([V|1]) so the
        AV matmul also produces the softmax denominator (row 64).
  exp   on ACT (scale=1/8 fused), causal mask = multiplicative f32 tile
        built on gpsimd; only lower-triangle blocks computed.
  normalize: reciprocal on DVE + PE ones-broadcast + DVE multiply.

Emission order is tuned so PE never starves: per token-quarter we do
qkv m-tiles, V blocks, the previous quarter's output projection, then
the attention i-loop with QK running 4 blocks ahead of AV.
"""
import sys

sys.path.insert(0, "/opt/trn_rl_repo")

import numpy as np

import concourse.bass as bass  # noqa: F401
import concourse.mybir as mybir
import concourse.tile as tile
from concourse import bacc
from concourse.bass_utils import run_bass_kernel_spmd

F32 = mybir.dt.float32
F32R = mybir.dt.float32r
Exp = mybir.ActivationFunctionType.Exp
Ident = mybir.ActivationFunctionType.Identity

B = 2
N = 2048
D = 1024
NH = 16
HD = 64
NCORES = 8
GROUPS = 4                # head groups (cores per batch)
HPC = NH // GROUPS        # heads per core = 4
PAIRS = HPC // 2          # head pairs per core = 2
QS = 512                  # q_super width
NQS = N // QS             # 4
NB = N // 128             # 16 token blocks
CCH = D // 128            # 8 contraction chunks

_CACHE = {}


def _build():
    nc = bacc.Bacc("TRN2", target_bir_lowering=False, debug=False,
                   num_devices=NCORES)
    xT = nc.dram_tensor("xT", [D, N], F32R, kind="ExternalInput").ap()
    W = nc.dram_tensor("W", [D, 768], F32R, kind="ExternalInput").ap()
    Wp = nc.dram_tensor("Wp", [256, D], F32R, kind="ExternalInput").ap()
    biasqk = nc.dram_tensor("biasqk", [128, 4], F32, kind="ExternalInput").ap()
    vbias = nc.dram_tensor("vbias", [128, 256], F32, kind="ExternalInput").ap()
    onesc = nc.dram_tensor("onesc", [128, NB * HPC], F32R, kind="ExternalInput").ap()
    ones64D = nc.dram_tensor("ones64D", [1, 64], F32R, kind="ExternalInput").ap()
    y = nc.dram_tensor("y", [N, D], F32, kind="ExternalOutput").ap()

    with tile.TileContext(nc) as tc:
        with (
            tc.tile_pool(name="persist", bufs=1) as pp,
            tc.tile_pool(name="xtq_pool", bufs=2) as xtq_pool,
            tc.tile_pool(name="et_pool", bufs=7) as et_pool,
            tc.tile_pool(name="work", bufs=2) as work,
            tc.tile_pool(name="ysb_pool", bufs=4) as ysb_pool,
            tc.tile_pool(name="mm", bufs=2, space="PSUM") as mm,
            tc.tile_pool(name="spool", bufs=2, space="PSUM") as spool,
            tc.tile_pool(name="opool", bufs=2, space="PSUM") as opool,
        ):
            # ---- persistent tiles ----
            W_sb = pp.tile([128, CCH, 768], F32R)
            Wp_sb = pp.tile([128, 2, D], F32R)
            bqk_sb = pp.tile([128, 4], F32)
            vbias_sb = pp.tile([128, 256], F32)
            ones64 = pp.tile([1, 64], F32R)
            qT = pp.tile([128, PAIRS, N], F32R)
            kT = pp.tile([128, PAIRS, N], F32R)
            onT = pp.tile([128, PAIRS, N], F32R)
            vaug = pp.tile([128, NB, HPC * 65], F32R)
            vaug_h = vaug.rearrange("p b (h c) -> p b h c", c=65)
            masks = pp.tile([128, 4, QS], F32)

            W_r = W.rearrange("(c p) f -> p c f", p=128)
            Wp_r = Wp.rearrange("(c p) f -> p c f", p=128)
            xT_r = xT.rearrange("(c p) n -> p c n", p=128)
            y_r = y.rearrange("(t p) f -> t p f", p=128)

            # causal masks on gpsimd (off the DMA critical path):
            # masks[p, t, q] = 1.0 iff q - p - 128*t >= 0
            nc.gpsimd.memset(masks[:], 1.0)
            for t in range(4):
                nc.gpsimd.affine_select(
                    out=masks[:, t, :],
                    in_=masks[:, t, :],
                    compare_op=mybir.AluOpType.is_ge,
                    fill=0.0,
                    base=-128 * t,
                    pattern=[[1, QS]],
                    channel_multiplier=-1,
                )

            pending_norm = []

            def emit_norm():
                """normalize deferred (j, hp, osb) entries: overlap the DVE
                reciprocal chain with the next quarter's PE work"""
                while pending_norm:
                    j, hp, osb = pending_norm.pop(0)
                    for h in range(2):
                        pb = 64 * h
                        recip = work.tile([1, QS], F32R, tag="recip",
                                          name=f"r{j}{hp}{h}")
                        with nc.allow_low_precision("f32r recip for PE bcast"):
                            nc.vector.reciprocal(recip[:], osb[h][64:65, :])
                        bc_ps = mm.tile([64, QS], F32, tag="mm",
                                        name=f"bc{j}{hp}{h}")
                        nc.tensor.matmul(bc_ps[:], ones64[:], recip[:],
                                         start=True, stop=True)
                        bc_sb = work.tile([64, QS], F32, tag="bc_sb",
                                          name=f"bs{j}{hp}{h}")
                        nc.scalar.copy(bc_sb[:], bc_ps[:])
                        nc.vector.tensor_mul(
                            onT[pb : pb + 64, hp, QS * j : QS * (j + 1)],
                            osb[h][0:64, :],
                            bc_sb[:],
                        )

            def emit_proj(jj, tail=False):
                """output projection for token quarter jj"""
                for blk in range(4):
                    tb = 4 * jj + blk
                    for nh in range(2):
                        yps = mm.tile([128, QS], F32, tag="mm",
                                      name=f"y{tb}{nh}")
                        for c in range(2):
                            nc.tensor.matmul(
                                yps[:],
                                onT[:, c, 128 * tb : 128 * (tb + 1)],
                                Wp_sb[:, c, QS * nh : QS * (nh + 1)],
                                start=(c == 0),
                                stop=(c == 1),
                            )
                        ysb = ysb_pool.tile([128, QS], F32, tag="ysb",
                                            name=f"ysb{tb}{nh}")
                        # tail: ACT is idle — alternate copy engines
                        if tail and (blk + nh) % 2 == 1:
                            nc.scalar.copy(ysb[:], yps[:])
                        else:
                            nc.vector.tensor_copy(ysb[:], yps[:])
                        nc.sync.dma_start(
                            y_r[tb][:, QS * nh : QS * (nh + 1)], ysb[:]
                        )

            def fetch_xq(q):
                t0, t1 = QS * q, QS * (q + 1)
                xq = xtq_pool.tile([128, CCH, QS], F32R, tag="xq",
                                   name=f"xq{q}")
                for ci in range(CCH):
                    nc.sync.dma_start(xq[:, ci, :], xT_r[:, ci, t0:t1])
                return xq

            next_xq = None
            for qtr in range(NQS):
                ts, te = QS * qtr, QS * (qtr + 1)
                j = qtr

                # ---- input DMAs, ordered by first consumption ----
                if qtr == 0:
                    xq = xtq_pool.tile([128, CCH, QS], F32R, tag="xq",
                                       name="xq0")
                    for ci in range(CCH):
                        nc.sync.dma_start(W_sb[:, ci, :], W_r[:, ci, :])
                        nc.sync.dma_start(xq[:, ci, :], xT_r[:, ci, ts:te])
                    nc.sync.dma_start(bqk_sb[:], biasqk)
                    nc.sync.dma_start(
                        vaug_h[:, :, :, 64],
                        onesc.rearrange("p (b h) -> p b h", h=HPC),
                    )
                    nc.sync.dma_start(ones64[:], ones64D)
                    nc.sync.dma_start(vbias_sb[:], vbias)
                    next_xq = fetch_xq(1)
                    for c in range(2):
                        nc.sync.dma_start(Wp_sb[:, c, :], Wp_r[:, c, :])
                else:
                    xq = next_xq
                    if qtr + 1 < NQS:
                        next_xq = fetch_xq(qtr + 1)

                # ---- qkv projection: q/k feature-major m-tiles ----
                # chunk-outer so quarter 0 consumes x chunks as they arrive
                for half in ((0, 1), (2, 3)):
                    ps = {
                        m: mm.tile([128, QS], F32, tag="mm", name=f"qk{qtr}{m}")
                        for m in half
                    }
                    for ci in range(CCH):
                        for m in half:
                            nc.tensor.matmul(
                                ps[m][:],
                                W_sb[:, ci, 128 * m : 128 * (m + 1)],
                                xq[:, ci, :],
                                start=(ci == 0),
                                stop=(ci == CCH - 1),
                            )
                    for m in half:
                        dst = qT if m < 2 else kT
                        nc.vector.tensor_scalar_add(
                            dst[:, m % 2, ts:te], ps[m][:], bqk_sb[:, m : m + 1]
                        )

                # ---- V token-major (with bias) into [V|1] slots ----
                for blk in range(4):
                    tb = 4 * qtr + blk
                    vps = mm.tile([128, 256], F32, tag="mm", name=f"v{qtr}{blk}")
                    for ci in range(CCH):
                        nc.tensor.matmul(
                            vps[:],
                            xq[:, ci, 128 * blk : 128 * (blk + 1)],
                            W_sb[:, ci, 512:768],
                            start=(ci == 0),
                            stop=(ci == CCH - 1),
                        )
                    nc.vector.tensor_add(
                        vaug_h[:, tb, :, 0:64],
                        vps.rearrange("p (h c) -> p h c", c=64),
                        vbias_sb.rearrange("p (h c) -> p h c", c=64),
                    )

                # previous quarter's normalize + proj fill PE/DVE while this
                # quarter's qT/kT copies and first exps complete
                if qtr > 0:
                    emit_norm()
                    emit_proj(qtr - 1)

                # ---- attention for q_super j ----
                n_i = 4 * j + 4
                for hp in range(PAIRS):
                    # flush hp0's normalize on the last quarter only (no next
                    # quarter to absorb it); mid-kernel it steals ACT/DVE from
                    # the exp pipeline
                    if qtr == NQS - 1:
                        emit_norm()
                    o_ps = {
                        h: opool.tile([65, QS], F32, tag="o", name=f"o{j}{hp}{h}")
                        for h in range(2)
                    }
                    ets = {}

                    def emit_qk(i):
                        t = i - 4 * j
                        qs0 = max(0, 128 * t)
                        sps = spool.tile([128, 2, QS], F32, tag="s",
                                         name=f"s{j}{hp}{i}")
                        for h in range(2):
                            pb = 64 * h
                            nc.tensor.matmul(
                                sps[:, h, qs0:],
                                kT[pb : pb + 64, hp, 128 * i : 128 * (i + 1)],
                                qT[pb : pb + 64, hp, QS * j + qs0 : QS * (j + 1)],
                                start=True,
                                stop=True,
                            )
                        et = et_pool.tile([128, 2, QS], F32R, tag="et",
                                          name=f"et{j}{hp}{i}")
                        nc.scalar.activation(
                            et[:, :, qs0:], sps[:, :, qs0:], Exp, scale=0.125,
                        )
                        if t >= 0:
                            # only the first 128 valid columns hold the triangle
                            nc.vector.tensor_mul(
                                et[:, :, qs0 : qs0 + 128],
                                et[:, :, qs0 : qs0 + 128],
                                masks[:, t, qs0 : qs0 + 128].unsqueeze(1)
                                .broadcast_to([128, 2, 128]),
                            )
                        ets[i] = et

                    def emit_av(i):
                        t = i - 4 * j
                        qs0 = max(0, 128 * t)
                        et = ets.pop(i)
                        for h in range(2):
                            hh = (2 * hp + h) * 65
                            nc.tensor.matmul(
                                o_ps[h][:, qs0:],
                                vaug[:, i, hh : hh + 65],
                                et[:, h, qs0:],
                                start=(i == 0),
                                stop=(i == n_i - 1),
                            )

                    LOOKAHEAD = 4
                    for i in range(n_i):
                        emit_qk(i)
                        if i >= LOOKAHEAD:
                            emit_av(i - LOOKAHEAD)
                    for i in range(max(0, n_i - LOOKAHEAD), n_i):
                        emit_av(i)

                    # drain o to SBUF (DVE is idle mid-attention) to free PSUM
                    osb = {}
                    for h in range(2):
                        osb[h] = work.tile([65, QS], F32, tag="osb", bufs=4,
                                           name=f"osb{j}{hp}{h}")
                        nc.vector.tensor_copy(osb[h][:], o_ps[h][:])
                    pending_norm.append((j, hp, osb))

            emit_norm()
            emit_proj(NQS - 1, tail=True)

    nc.compile()
    return nc


def _host_prep(x, W_qkv, b_qkv, W_proj, b_proj):
    """Build per-core input maps."""
    x = np.asarray(x, dtype=np.float32)
    W_qkv = np.asarray(W_qkv, dtype=np.float32)
    b_qkv = np.asarray(b_qkv, dtype=np.float32)
    W_proj = np.asarray(W_proj, dtype=np.float32)

    onesc = np.ones((128, NB * HPC), dtype=np.float32)
    ones64D = np.ones((1, 64), dtype=np.float32)

    xTs = [np.ascontiguousarray(x[b].T) for b in range(B)]

    in_maps = []
    for c in range(NCORES):
        b, g = divmod(c, GROUPS)
        cols = slice(256 * g, 256 * (g + 1))
        Wslice = np.ascontiguousarray(
            np.concatenate(
                [W_qkv[:, cols], W_qkv[:, 1024:2048][:, cols],
                 W_qkv[:, 2048:3072][:, cols]],
                axis=1,
            )
        )
        bq = b_qkv[cols.start : cols.stop]
        bk = b_qkv[1024 + cols.start : 1024 + cols.stop]
        bv = b_qkv[2048 + cols.start : 2048 + cols.stop]
        biasqk = np.ascontiguousarray(
            np.stack([bq[:128], bq[128:], bk[:128], bk[128:]], axis=1)
        )
        vbias = np.ascontiguousarray(np.broadcast_to(bv, (128, 256)))
        Wp_slice = np.ascontiguousarray(W_proj[cols])
        in_maps.append(
            {
                "xT": xTs[b],
                "W": Wslice,
                "Wp": Wp_slice,
                "biasqk": biasqk,
                "vbias": vbias,
                "onesc": onesc,
                "ones64D": ones64D,
            }
        )
    return in_maps


def kernel(x, W_qkv, b_qkv, W_proj, b_proj):
    if "nc" not in _CACHE:
        _CACHE["nc"] = _build()
    nc = _CACHE["nc"]
    in_maps = _host_prep(x, W_qkv, b_qkv, W_proj, b_proj)
    res = run_bass_kernel_spmd(nc, in_maps, core_ids=list(range(NCORES)))
    out = np.zeros((B, N, D), dtype=np.float32)
    bp = np.asarray(b_proj, dtype=np.float32)
    for b in range(B):
        acc = res.results[4 * b]["y"].astype(np.float32).copy()
        for g in range(1, GROUPS):
            acc += res.results[4 * b + g]["y"]
        out[b] = acc + bp
    return out
